# revision 2
# baseline (speedup 1.0000x reference)
"""MobileMamba block kernel for 8x Trainium2 NeuronCores — v2.

Math restructure (same as v1):
  xc   = silu(x @ w1.T + b1)                          # [E, L] channel-major
  c    = depthwise_conv5(xc) (+bd, BN affine folded)
  xl   = silu(c)
  SSM with constant B/C collapses to a scalar first-order recurrence.
    We pre-scale:  xs = (CB/Dv) * xl   (DVE tensor_scalar, 4x mode)
    scan:          g' = a*g' + xs      (DVE tensor_tensor_scan)
    fold:          gp = g' + xl        (DVE tensor_tensor, 2x mode)
  out  = w2dv @ gp + b2   (w2dv = w2.T * Dv)

v2 changes vs v1 (driven by HW microbenchmarks):
  - conv tile 3 runs on DVE (ts + 4 stt, ping-pong, pair-granular 1024)
    instead of PE; tiles 0-2 stay as 5 PSUM-accumulated diag matmuls.
  - xc has 2 zero-pad columns around each tile so every conv tap is a
    full-width access (no edge clamping, no partial PSUM groups).
  - mm1 at pair (1024) granularity into 2-bank PSUM pairs, one 1024-wide
    silu per (tile, pair) on Act.
  - chunk-major pipeline; PE program order: all mm1, conv c0..c2,
    mm2 c0..c1, conv c3, mm2 c2..c3 (keeps PE dense and the tail short).
  - big flat SBUF tensors; weights DMAed in stripes so mm1 starts ~2us in.

Sharding: data-parallel over batch (B=8 -> 8 cores), one sample per core.
"""

import sys

for _p in ('/opt/trn_rl_repo',):
    if _p not in sys.path:
        sys.path.append(_p)

import numpy as np

import concourse.bass as bass
import concourse.tile as tile
from concourse import mybir

D = 256      # model dim
E = 512      # expanded dim
L = 2048     # sequence length
NCORES = 8
BN_EPS = 1e-5

F32 = mybir.dt.float32
BF16 = mybir.dt.bfloat16

EM = E // 128   # 4 channel tiles
DM = D // 128   # 2 model-dim tiles
CH = 512        # chunk
LC = L // CH    # 4 chunks
PR = 1024       # pair
NP = L // PR    # 2 pairs

TW = L + 4      # padded tile width in xc (2 zero cols each side)
TAPS = (-2, -1, 0, 1, 2)

# mpc (f32 per-channel param) columns
PC_B1 = 0        # 4 cols: b1 per tile
PC_CBIAS = 4     # 4 cols: conv+bn bias per tile
PC_CBDV = 8      # 4 cols: CB/Dv per tile
PC_EXPA = 12     # 4 cols: expA per tile
PC_CW = 16       # 20 cols: conv taps per (tile, tap) f32 (DVE conv)
PC_B2 = 36       # 2 cols: b2 per dt
PC_NCOL = 38

DVE_CONV_TILES = (3,)          # conv tiles computed on DVE
PE_CONV_TILES = (0, 1, 2)


def _bcast(col_ap, n):
    return bass.AP(tensor=col_ap.tensor, offset=col_ap.offset,
                   ap=[col_ap.ap[0], [0, n]])


def build_nc():
    nc = bass.Bass()
    xt = nc.declare_dram_parameter("xt", [D, L], BF16, isOutput=False)
    mw1 = nc.declare_dram_parameter("mw1", [128, DM * E], BF16, isOutput=False)
    mw2 = nc.declare_dram_parameter("mw2", [128, EM * D], BF16, isOutput=False)
    mdg = nc.declare_dram_parameter("mdg", [128, len(PE_CONV_TILES) * 5 * 128],
                                    BF16, isOutput=False)
    mae = nc.declare_dram_parameter("mae", [128, EM * CH], BF16, isOutput=False)
    mpc = nc.declare_dram_parameter("mpc", [128, PC_NCOL], F32, isOutput=False)
    outT = nc.declare_dram_parameter("outT", [D, L], F32, isOutput=True)

    with tile.TileContext(nc) as tc:
        with (
            tc.tile_pool(name="const", bufs=1) as const,
            tc.tile_pool(name="acts", bufs=1) as acts,
            tc.tile_pool(name="psA", bufs=2, space="PSUM") as psA,
            tc.tile_pool(name="psB", bufs=2, space="PSUM") as psB,
            tc.tile_pool(name="psC", bufs=2, space="PSUM") as psC,
        ):
            # ---------- constants / inputs ----------
            mpc_t = const.tile([128, PC_NCOL], F32)
            nc.sync.dma_start(out=mpc_t, in_=mpc[:, :])
            # w1t stripes: (k, m) so the first matmul can start early
            mw1_t = const.tile([128, DM * E], BF16)
            for k in range(DM):
                for m in range(EM):
                    c0 = k * E + m * 128
                    nc.sync.dma_start(out=mw1_t[:, c0:c0 + 128],
                                      in_=mw1[:, c0:c0 + 128])
            # x stripes, chunk-major, k inner; chunk 0 split in 2 for latency
            xts = const.tile([128, DM * L], BF16)  # k-tile k at cols k*L
            for k in range(DM):
                for h in range(2):
                    nc.sync.dma_start(
                        out=xts[:, k * L + h * 256:k * L + (h + 1) * 256],
                        in_=xt[k * 128:(k + 1) * 128, h * 256:(h + 1) * 256])
            for lc in range(1, LC):
                for k in range(DM):
                    nc.sync.dma_start(
                        out=xts[:, k * L + lc * CH:k * L + (lc + 1) * CH],
                        in_=xt[k * 128:(k + 1) * 128, lc * CH:(lc + 1) * CH])
            mdg_t = const.tile([128, len(PE_CONV_TILES) * 5 * 128], BF16)
            nc.gpsimd.dma_start(out=mdg_t, in_=mdg[:, :])
            mae_t = const.tile([128, EM * CH], BF16)
            nc.gpsimd.dma_start(out=mae_t, in_=mae[:, :])
            mw2_t = const.tile([128, EM * D], BF16)
            nc.gpsimd.dma_start(out=mw2_t, in_=mw2[:, :])

            # ---------- SBUF activations (flat tensors) ----------
            xc = acts.tile([128, EM * TW], BF16, name="xc", tag="xc")
            xl = acts.tile([128, EM * L], BF16, name="xl", tag="xl")
            xs = acts.tile([128, EM * L], BF16, name="xs", tag="xs")
            g = acts.tile([128, EM * L], BF16, name="g", tag="g")
            gp = acts.tile([128, EM * L], BF16, name="gp", tag="gp")
            osb = acts.tile([128, DM * L], F32, name="osb", tag="osb")
            cp0 = acts.tile([128, PR], BF16, name="cp0", tag="cp0")
            cp1 = acts.tile([128, PR], BF16, name="cp1", tag="cp1")

            # zero the xc pad columns (2 each side per tile)
            for m in range(EM):
                nc.gpsimd.memset(xc[:, m * TW:m * TW + 2], 0.0)
                nc.gpsimd.memset(xc[:, m * TW + 2 + L:(m + 1) * TW], 0.0)

            # ---------- per-engine touches (collapse DMA sem waits) ----------
            ps_scr = psA.tile([128, 8], F32, name="ps_scr", tag="psA")
            nc.tensor.matmul(out=ps_scr[:, 0:4], lhsT=mw1_t[:, 0:128],
                             rhs=mw1_t[:, 0:4], start=True, stop=True)
            v_scr = const.tile([128, 1], F32)
            nc.vector.tensor_copy(out=v_scr, in_=mpc_t[:, 0:1])
            a_scr = const.tile([128, 1], F32)
            nc.scalar.copy(out=a_scr, in_=mpc_t[:, 0:1])
            g_scr = const.tile([128, 1], F32)
            nc.gpsimd.tensor_copy(out=g_scr, in_=mpc_t[:, 0:1])

            # ---------- helpers ----------
            def xc_ap(m, t0, n):
                """AP into padded xc for tile m, logical time t0 (may be -2..L+2)."""
                return xc[:, m * TW + 2 + t0:m * TW + 2 + t0 + n]

            def pcol(c):
                return mpc_t[:, c:c + 1]

            w1s = [mw1_t[:, k * E:(k + 1) * E] for k in range(DM)]

            def mm1_pair(m, p):
                psp = psA.tile([128, PR], F32, name="ps1", tag="psA")
                for h in range(2):
                    c0 = p * PR + h * CH
                    for k in range(DM):
                        nc.tensor.matmul(
                            out=psp[:, h * CH:(h + 1) * CH],
                            lhsT=w1s[k][:, m * 128:(m + 1) * 128],
                            rhs=xts[:, k * L + c0:k * L + c0 + CH],
                            start=(k == 0), stop=(k == DM - 1))
                nc.scalar.activation(
                    out=xc_ap(m, p * PR, PR), in_=psp,
                    func=mybir.ActivationFunctionType.Silu,
                    bias=pcol(PC_B1 + m), scale=1.0)

            def conv_pe(m, lc):
                """5 diag matmuls into PSUM + 512-wide silu2."""
                mi = PE_CONV_TILES.index(m)
                a0 = lc * CH
                ps2 = psB.tile([128, CH], F32, name="ps2", tag="psB")
                for j, dlt in enumerate(TAPS):
                    dg = mdg_t[:, (mi * 5 + j) * 128:(mi * 5 + j + 1) * 128]
                    nc.tensor.matmul(
                        out=ps2, lhsT=dg, rhs=xc_ap(m, a0 + dlt, CH),
                        start=(j == 0), stop=(j == len(TAPS) - 1))
                nc.scalar.activation(
                    out=xl[:, m * L + a0:m * L + a0 + CH], in_=ps2,
                    func=mybir.ActivationFunctionType.Silu,
                    bias=pcol(PC_CBIAS + m), scale=1.0)

            def conv_dve(m, p):
                """ts + 4 stt ping-pong over pair p; then 2x 512-wide silu2."""
                a0 = p * PR
                bufs = (cp0, cp1)
                # center tap first (full range incl pad reads)
                nc.vector.tensor_scalar(
                    out=bufs[0], in0=xc_ap(m, a0, PR),
                    scalar1=pcol(PC_CW + m * 5 + 2), scalar2=None,
                    op0=mybir.AluOpType.mult)
                src = 0
                for j, dlt in enumerate((-2, -1, 1, 2)):
                    ji = dlt + 2
                    nc.vector.scalar_tensor_tensor(
                        out=bufs[1 - src], in0=xc_ap(m, a0 + dlt, PR),
                        scalar=pcol(PC_CW + m * 5 + ji), in1=bufs[src],
                        op0=mybir.AluOpType.mult, op1=mybir.AluOpType.add)
                    src = 1 - src
                for h in range(2):
                    nc.scalar.activation(
                        out=xl[:, m * L + a0 + h * CH:m * L + a0 + (h + 1) * CH],
                        in_=bufs[src][:, h * CH:(h + 1) * CH],
                        func=mybir.ActivationFunctionType.Silu,
                        bias=pcol(PC_CBIAS + m), scale=1.0)

            def scan_path(m, lc):
                a0, b0 = m * L + lc * CH, m * L + (lc + 1) * CH
                nc.vector.tensor_scalar(
                    out=xs[:, a0:b0], in0=xl[:, a0:b0],
                    scalar1=pcol(PC_CBDV + m), scalar2=None,
                    op0=mybir.AluOpType.mult)
                nc.vector.tensor_tensor_scan(
                    out=g[:, a0:b0], data0=mae_t[:, m * CH:m * CH + CH],
                    data1=xs[:, a0:b0],
                    initial=(0.0 if lc == 0 else g[:, a0 - 1:a0]),
                    op0=mybir.AluOpType.mult, op1=mybir.AluOpType.add)
                nc.vector.tensor_tensor(
                    out=gp[:, a0:b0], in0=g[:, a0:b0], in1=xl[:, a0:b0],
                    op=mybir.AluOpType.add)

            def mm2_chunk(dt, lc):
                a0 = lc * CH
                ps3 = psC.tile([128, CH], F32, name="ps3", tag="psC")
                # DVE conv tiles accumulated last so mm2 can start before
                # their gp is ready
                order = list(PE_CONV_TILES) + list(DVE_CONV_TILES)
                for i, ec in enumerate(order):
                    nc.tensor.matmul(
                        out=ps3,
                        lhsT=mw2_t[:, ec * D + dt * 128:ec * D + (dt + 1) * 128],
                        rhs=gp[:, ec * L + a0:ec * L + a0 + CH],
                        start=(i == 0), stop=(i == len(order) - 1))
                nc.scalar.activation(
                    out=osb[:, dt * L + a0:dt * L + a0 + CH], in_=ps3,
                    func=mybir.ActivationFunctionType.Identity,
                    bias=pcol(PC_B2 + dt), scale=1.0)
                nc.gpsimd.dma_start(
                    out=outT[dt * 128:(dt + 1) * 128, a0:a0 + CH],
                    in_=osb[:, dt * L + a0:dt * L + a0 + CH])

            # ---------- emission (per-engine program order) ----------
            # PE: mm1 (pair-major), conv c0-c2, mm2 c0-c1, conv c3, mm2 c2-c3
            # DVE: conv3 p0, scans c0, c1, conv3 p1, scans c2, c3
            for p in range(NP):
                for m in range(EM):
                    mm1_pair(m, p)
            for m in DVE_CONV_TILES:
                conv_dve(m, 0)
            for lc in (0, 1):
                for m in PE_CONV_TILES:
                    conv_pe(m, lc)
                for m in range(EM):
                    scan_path(m, lc)
            for m in DVE_CONV_TILES:
                conv_dve(m, 1)
            for m in PE_CONV_TILES:
                conv_pe(m, 2)
            for m in range(EM):
                scan_path(m, 2)
            for dt in range(DM):
                mm2_chunk(dt, 0)
            for dt in range(DM):
                mm2_chunk(dt, 1)
            for m in PE_CONV_TILES:
                conv_pe(m, 3)
            for m in range(EM):
                scan_path(m, 3)
            for dt in range(DM):
                mm2_chunk(dt, 2)
            for dt in range(DM):
                mm2_chunk(dt, 3)

    _split_waits(nc)
    return nc


_WSPLIT_SKIP = ("InstAllEngineBarrier", "InstNoOp",
                "InstEventSemaphore", "InstUnconditionalBranch")


def _split_waits(nc, max_waits=1):
    """Walrus allows one sync-wait command per TPB instruction; spill extra
    waits onto same-engine NoOps."""
    n_split = 0
    for f in nc.m.functions:
        for bb in f.blocks:
            out = []
            for inst in bb.instructions:
                si = inst.sync_info
                waits = list(si.on_wait) if si and si.on_wait else []
                if (len(waits) > max_waits
                        and inst.__class__.__name__ not in _WSPLIT_SKIP):
                    spill, keep = waits[:-max_waits], waits[-max_waits:]
                    for i, w in enumerate(spill):
                        out.append(mybir.InstNoOp(
                            name=f"{inst.name}_ws{i}",
                            engine=inst.engine,
                            sync_info=mybir.SyncInfo(on_wait=[w],
                                                     on_update=[]),
                        ))
                        n_split += 1
                    si.on_wait = keep
                out.append(inst)
            if n_split:
                bb.instructions = out
    return nc


def _to_bf16(a):
    import ml_dtypes
    return np.asarray(a, np.float32).astype(ml_dtypes.bfloat16)


def host_params(w1, b1, wd, bd, gamma, beta, rmean, rvar, A, Bm, Cm, Dv, w2, b2):
    s = (gamma / np.sqrt(rvar + BN_EPS)).astype(np.float32)
    cw = (wd[:, 0, :] * s[:, None]).astype(np.float32)            # [E, 5]
    cbias = (bd * s + beta - rmean * s).astype(np.float32)        # [E]
    expA = np.exp(np.asarray(A, np.float32))                      # [E]
    CB = (np.asarray(Bm, np.float32) * np.asarray(Cm, np.float32)).sum(1)
    w1t = np.asarray(w1, np.float32).T                            # [D, E]
    w2t = np.asarray(w2, np.float32).T                            # [E, D]

    dv = np.asarray(Dv, np.float32).copy()
    tiny = np.abs(dv) < 1e-6
    dv[tiny] = np.where(dv[tiny] < 0, -1e-6, 1e-6)
    cbdv = (CB / dv).astype(np.float32)

    mw1 = np.zeros((128, DM * E), np.float32)
    for k in range(DM):
        mw1[:, k * E:(k + 1) * E] = w1t[k * 128:(k + 1) * 128, :]

    mw2 = np.zeros((128, EM * D), np.float32)
    for ec in range(EM):
        mw2[:, ec * D:(ec + 1) * D] = \
            w2t[ec * 128:(ec + 1) * 128, :] * dv[ec * 128:(ec + 1) * 128, None]

    mdg = np.zeros((128, len(PE_CONV_TILES) * 5 * 128), np.float32)
    for mi, m in enumerate(PE_CONV_TILES):
        for j in range(5):
            blk = np.zeros((128, 128), np.float32)
            np.fill_diagonal(blk, cw[m * 128:(m + 1) * 128, j])
            mdg[:, (mi * 5 + j) * 128:(mi * 5 + j + 1) * 128] = blk

    mae = np.zeros((128, EM * CH), np.float32)
    for m in range(EM):
        mae[:, m * CH:(m + 1) * CH] = expA[m * 128:(m + 1) * 128, None]

    mpc = np.zeros((128, PC_NCOL), np.float32)
    for m in range(EM):
        sl = slice(m * 128, (m + 1) * 128)
        mpc[:, PC_B1 + m] = np.asarray(b1, np.float32)[sl]
        mpc[:, PC_CBIAS + m] = cbias[sl]
        mpc[:, PC_CBDV + m] = cbdv[sl]
        mpc[:, PC_EXPA + m] = expA[sl]
        for j in range(5):
            mpc[:, PC_CW + m * 5 + j] = cw[sl, j]
    for dt in range(DM):
        mpc[:, PC_B2 + dt] = np.asarray(b2, np.float32)[dt * 128:(dt + 1) * 128]

    return dict(mw1=_to_bf16(mw1), mw2=_to_bf16(mw2), mdg=_to_bf16(mdg),
                mae=_to_bf16(mae), mpc=mpc)


_CACHED_NC = None


def kernel(x, w1, b1, wd, bd, gamma, beta, rmean, rvar, A, Bm, Cm, Dv, w2, b2,
           **run_kwargs):
    from concourse.bass_utils import run_bass_kernel_spmd
    global _CACHED_NC
    if _CACHED_NC is None:
        _CACHED_NC = build_nc()
    nc = _CACHED_NC

    params = host_params(w1, b1, wd, bd, gamma, beta, rmean, rvar,
                         A, Bm, Cm, Dv, w2, b2)
    x = np.asarray(x, dtype=np.float32)
    in_maps = []
    for i in range(NCORES):
        m = dict(params)
        m["xt"] = _to_bf16(np.ascontiguousarray(x[i].T))  # [D, L] bf16
        in_maps.append(m)

    res = run_bass_kernel_spmd(nc, in_maps, core_ids=list(range(NCORES)),
                               **run_kwargs)
    out = np.stack([np.asarray(r["outT"]).T for r in res.results])  # [B, L, D]
    if run_kwargs:
        kernel.last_result = res
    return out
